# revision 36
# baseline (speedup 1.0000x reference)
"""GaussianImage (Cholesky) renderer on 8 trn2 NeuronCores.

Tile-parallel over the pixel grid: the 256x256 image is cut into 32x32
tiles (64/frame, 128 total).  The host bins gaussians to tiles (pure
routing via a conservative support radius), then pours the tile lists
into sixteen 128-slot "groups", SPLITTING a tile across groups when one
fills -- rendering is an additive accumulation, so partial tile images
are summed on the host, and the final clamp happens post-gather (the
clamp must follow the reduce).  Each core runs exactly 2 groups (slot
sets), each rendered as 2 pixel-half units of [128 slots x 512 px].
Per-core params / masks / outputs are DATA, so the SPMD program is
identical on every core.  All value math runs on device:

  per gaussian slot : tanh / exp-sigmoid / conic / quadratic-basis
                      coeffs split across Scalar(ACT)/Vector/GpSimd
  per slot set      : coeffs hi/lo-split in the FREE dim as bf16, one
                      PE transpose -> stacked [12,128] lhsT, so sigma
                      is ONE matmul with contraction 12 (the hi+lo
                      accumulation rides the contraction dim for free;
                      fp32-level coefficients at bf16 matmul rate)
  per unit          : sigma = chilo(12,128)^T @ basis(12,512)  [TensorE]
                      alpha = Exp(-sigma)           -> bf16    [ScalarE]
                      img   = wblk(128,3K)^T @ alpha(128,512)  [TensorE]
                      out   = bf16 copy to SBUF, DMA out unclamped

Single ACT table set: everything Scalar runs (exp/tanh/square/copy)
lives in the `exp_and_others` table, so exactly one ACT_TABLE_LOAD is
issued, and it loads eagerly during the input DMAs (the baseline paid
two 1.3us loads).  Sigmoids for opacity/color are computed as
1/((1+e^-o)(1+e^-c)) via Exp + Vector reciprocal, folding the
opacity*color product into the same reciprocal.  Input DMAs are split
across the sync (params, basis) and gpsimd (msk, ident) queues so the
params DMA -- which gates all setup math -- issues first; outputs are
bf16 and spread over sync/gpsimd/scalar queues.
"""

import os
import numpy as np
import ml_dtypes

T, N, H, W = 2, 512, 256, 256
TILE = 16
NT = H // TILE          # 16 tiles per axis
N_CORES = 8
SLOTS = 128
PIX = TILE * TILE       # 256
SIGMA_CUT = 8.0         # exp(-8)*opac ~ 2e-4 tail: invisible at 2e-2 tolerance

_CACHE = {}


def _build_nc(U, SETS, K):
    """U pixel-units (512 cols each) over SETS slot-sets per core.

    Unit u reads slot-set u//2 and pixel-half u%2; params, masks and
    outputs are per-core data, so the SPMD program is identical on every
    core while cores render different (gaussian-set, pixel-half) units."""
    import concourse.bass as bass
    import concourse.mybir as mybir
    from concourse.tile import TileContext
    import bass_rust

    f32 = mybir.dt.float32
    bf16 = mybir.dt.bfloat16
    Alu = mybir.AluOpType
    Act = mybir.ActivationFunctionType

    HCOL = PIX              # 256: one unit = one slot-set over a full tile
    NB = 10                 # basis rows: x2 xy y2 x x y y 1 1 1
    nc = bass.Bass("TRN2")
    params = nc.dram_tensor("params", [SLOTS, SETS * 12], f32, kind="ExternalInput")
    basis = nc.dram_tensor("basis", [2 * NB, HCOL], bf16, kind="ExternalInput")
    msk = nc.dram_tensor("msk", [SLOTS, SETS * 3 * K], bf16, kind="ExternalInput")
    ident = nc.dram_tensor("ident", [SLOTS, SLOTS], bf16, kind="ExternalInput")
    out = nc.dram_tensor("out", [3 * K, U * HCOL], bf16, kind="ExternalOutput")

    with TileContext(nc) as tc:
        with tc.tile_pool(name="const", bufs=1) as cpool, \
             tc.tile_pool(name="work", bufs=5) as wpool, \
             tc.tile_pool(name="ps_sig", bufs=4, space="PSUM") as ps_sig_pool, \
             tc.tile_pool(name="ps_img", bufs=4, space="PSUM") as ps_img_pool:

            p3 = cpool.tile([SLOTS, SETS, 12], f32, tag="params")
            bt = cpool.tile([2 * NB, HCOL], bf16, tag="basis")
            mt = cpool.tile([SLOTS, SETS, 3 * K], bf16, tag="msk")
            it = cpool.tile([SLOTS, SLOTS], bf16, tag="ident")
            # Every DMA is shredded into one ring descriptor per partition row
            # (~9ns each, globally dispatched in issue order after a ~770ns
            # ring-kick lag), so params -- which gates ALL setup math -- is
            # issued FIRST and split across the two hwdge queues to halve its
            # descriptor tail.  The rest follows; msk/ident/basis are only
            # consumed ~2-3us later.
            HP = SLOTS // 2
            pview = params[:].rearrange("p (g k) -> p g k", k=12)
            nc.sync.dma_start(out=p3[0:HP], in_=pview[0:HP])
            nc.scalar.dma_start(out=p3[HP:SLOTS], in_=pview[HP:SLOTS])
            nc.sync.dma_start(out=mt, in_=msk[:].rearrange("p (g k) -> p g k", k=3 * K))
            nc.sync.dma_start(out=bt, in_=basis[:])
            # ident rides the gpsimd (softdge) ring, issued LAST so its 128
            # descriptors dispatch after everything else -- and the doorbell
            # pre-arms the gpsimd ring, whose first kick otherwise costs the
            # output DMAs ~600ns at the tail.
            nc.gpsimd.dma_start(out=it, in_=ident[:])

            V = nc.vector
            S = nc.scalar
            GP = nc.gpsimd

            # hf doubles as the Cholesky +0.5 bound and the warm-up input.
            hf = cpool.tile([SLOTS, 1], f32, tag="hf")
            GP.memset(hf, 0.5)
            # warm-up exp: zero-dependency first ACT op.  Every function the
            # Scalar engine runs (exp/tanh/square/copy) is in the
            # exp_and_others table set, so this triggers the ONLY table load
            # of the kernel, overlapping the input DMAs.
            warm = cpool.tile([SLOTS, 1], f32, tag="warm")
            S.activation(warm, hf, Act.Exp)

            def sc(tag):
                return cpool.tile([SLOTS, SETS], f32, tag=tag, name=tag)

            cth = cpool.tile([SLOTS, SETS, NB], f32, tag="coef")

            # --- critical setup chain in natural (early) priority; the
            # off-critical color/opacity block below is pushed to a very LATE
            # priority so the list scheduler never wedges it (w3's reciprocal
            # especially) into an idle slot AHEAD of this chain.
            tx = cpool.tile([SLOTS, SETS, 2], f32, tag="tx")
            t2, t3 = sc("t2"), sc("t3")
            cc = cpool.tile([SLOTS, SETS, 2 * NB], bf16, tag="cc")
            if True:
                # Scalar: tanh centers + the squares gating the det chain
                S.activation(tx, p3[:, :, 0:2], Act.Tanh)
                S.activation(t2, p3[:, :, 3], Act.Square)
                S.activation(t3, p3[:, :, 4], Act.Square, bias=hf)

                # GpSimd: head of the det chain
                a0 = sc("a0")
                GP.tensor_add(out=a0, in0=p3[:, :, 2],
                              in1=hf.broadcast_to([SLOTS, SETS]))
                a1 = p3[:, :, 3]
                t0, t1, v = sc("t0"), sc("t1"), sc("v")
                GP.tensor_mul(out=t0, in0=a0, in1=a0)
                GP.tensor_mul(out=t1, in0=a0, in1=a1)
                GP.tensor_mul(out=v, in0=t1, in1=t1)

                # Vector: centers, det -> rdet -> conic coeffs
                # cx = 0.5*W*(tanh(z)+1); host bakes p9 = tile_cx - 0.5*W so
                # ex = 0.5*W*tanh - p9.
                ex, ey = sc("ex"), sc("ey")
                V.scalar_tensor_tensor(out=ex, in0=tx[:, :, 0], scalar=0.5 * W,
                                       in1=p3[:, :, 9], op0=Alu.mult,
                                       op1=Alu.subtract)
                V.scalar_tensor_tensor(out=ey, in0=tx[:, :, 1], scalar=0.5 * H,
                                       in1=p3[:, :, 10], op0=Alu.mult,
                                       op1=Alu.subtract)
                syy = sc("syy")
                V.tensor_add(out=syy, in0=t2, in1=t3)
                u_, det, rdet = sc("u"), sc("det"), sc("rdet")
                V.tensor_mul(out=u_, in0=t0, in1=syy)
                V.tensor_sub(out=det, in0=u_, in1=v)
                V.reciprocal(out=rdet, in_=det)
                # cth0 = 0.5*ca, cth1 = cb = -sxy/det, cth2 = 0.5*cc
                V.scalar_tensor_tensor(out=cth[:, :, 0], in0=syy, scalar=0.5,
                                       in1=rdet, op0=Alu.mult, op1=Alu.mult)
                V.scalar_tensor_tensor(out=cth[:, :, 1], in0=t1, scalar=-1.0,
                                       in1=rdet, op0=Alu.mult, op1=Alu.mult)
                V.scalar_tensor_tensor(out=cth[:, :, 2], in0=t0, scalar=0.5,
                                       in1=rdet, op0=Alu.mult, op1=Alu.mult)
                # The 10-row basis [x2 xy y2 x x y y 1 1 1] lets every
                # remaining coefficient be ONE fused stt op at depth <= 2 past
                # cth0-2 (instead of the depth-5 m/n accumulation chains); the
                # extra rows ride the matmul contraction dim for free.
                #   x rows: p_a = -ca*ex        p_b = -cb*ey
                #   y rows: p_c = -cc*ey        p_d = -cb*ex
                #   1 rows: q_a = .5*ca*ex^2    q_b = cb*ex*ey  q_c = .5*cc*ey^2
                # (TensorScalarPtr only exists on DVE, so all seven sit on V.)
                V.scalar_tensor_tensor(out=cth[:, :, 3], in0=ex, scalar=-2.0,
                                       in1=cth[:, :, 0], op0=Alu.mult, op1=Alu.mult)
                V.scalar_tensor_tensor(out=cth[:, :, 4], in0=ey, scalar=-1.0,
                                       in1=cth[:, :, 1], op0=Alu.mult, op1=Alu.mult)
                V.scalar_tensor_tensor(out=cth[:, :, 5], in0=ey, scalar=-2.0,
                                       in1=cth[:, :, 2], op0=Alu.mult, op1=Alu.mult)
                V.scalar_tensor_tensor(out=cth[:, :, 6], in0=ex, scalar=-1.0,
                                       in1=cth[:, :, 1], op0=Alu.mult, op1=Alu.mult)
                V.scalar_tensor_tensor(out=cth[:, :, 7], in0=ex, scalar=-0.5,
                                       in1=cth[:, :, 3], op0=Alu.mult, op1=Alu.mult)
                V.scalar_tensor_tensor(out=cth[:, :, 8], in0=ey, scalar=-1.0,
                                       in1=cth[:, :, 6], op0=Alu.mult, op1=Alu.mult)
                V.scalar_tensor_tensor(out=cth[:, :, 9], in0=ey, scalar=-0.5,
                                       in1=cth[:, :, 5], op0=Alu.mult, op1=Alu.mult)

                # hi/lo split IN THE FREE DIM as bf16, then one PE transpose
                # per set yields a stacked [20,128] lhsT: sigma rides the
                # contraction dim (free on the PE) instead of a 2nd matmul.
                V.tensor_copy(out=cc[:, :, 0:NB], in_=cth)
                V.tensor_sub(out=cc[:, :, NB:2 * NB], in0=cth, in1=cc[:, :, 0:NB])

                chilos = []
                for g in range(SETS):
                    tpg = ps_img_pool.tile([2 * NB, SLOTS], bf16, tag="img",
                                           name=f"tp{g}")
                    nc.tensor.transpose(tpg, cc[:, g, :], it)
                    chilo = cpool.tile([2 * NB, SLOTS], bf16, tag=f"chilo{g}",
                                       name=f"chilo{g}")
                    # copies alternate engines so consecutive sets land in
                    # parallel; set 0 rides Vector (its copy is ~150ns
                    # faster) because it gates the first sigma matmul.
                    if g % 2 == 0:
                        V.tensor_copy(out=chilo, in_=tpg)
                    else:
                        S.copy(out=chilo, in_=tpg)
                    chilos.append(chilo)

            # --- off-critical-path setup at VERY LATE priority (loses every
            # ready-tie, still fills genuinely idle slots): colors*opacity
            # without the sigmoid table: sigmoid(o)*sigmoid(c) =
            # 1/((1+e^-o)(1+e^-c)) -- one reciprocal.
            with tc.high_priority(offset=-100000):
                eo = sc("eo")
                S.activation(eo, p3[:, :, 5], Act.Exp, scale=-1.0)
                ew = cpool.tile([SLOTS, SETS, 3], f32, tag="ew")
                S.activation(ew, p3[:, :, 6:9], Act.Exp, scale=-1.0)
                deno = sc("deno")
                GP.tensor_scalar_add(deno, eo, 1.0)
                denw = cpool.tile([SLOTS, SETS, 3], f32, tag="denw")
                GP.tensor_scalar_add(denw, ew, 1.0)
                prod = cpool.tile([SLOTS, SETS, 3], f32, tag="prodw")
                GP.tensor_mul(out=prod, in0=denw,
                              in1=deno.unsqueeze(2).broadcast_to([SLOTS, SETS, 3]))
                # 1/prod on GpSimd (bit-trick seed + 2 Newton steps, rel err
                # ~1e-6): keeps the reciprocal OFF Vector, whose static
                # schedule otherwise stalls the det chain waiting for prod.
                u32 = mybir.dt.uint32
                magic = cpool.tile([SLOTS, 1], u32, tag="magic")
                GP.memset(magic, 0x7EF127EA)
                two = cpool.tile([SLOTS, 1], f32, tag="two")
                GP.memset(two, 2.0)
                w3 = cpool.tile([SLOTS, SETS, 3], f32, tag="w3")
                r0 = cpool.tile([SLOTS, SETS, 3], f32, tag="w3r0")
                r1 = cpool.tile([SLOTS, SETS, 3], f32, tag="w3r1")
                n1 = cpool.tile([SLOTS, SETS, 3], f32, tag="w3n1")
                n2 = cpool.tile([SLOTS, SETS, 3], f32, tag="w3n2")
                n3 = cpool.tile([SLOTS, SETS, 3], f32, tag="w3n3")
                n4 = cpool.tile([SLOTS, SETS, 3], f32, tag="w3n4")
                mb = magic.unsqueeze(2).broadcast_to([SLOTS, SETS, 3])
                tb = two.unsqueeze(2).broadcast_to([SLOTS, SETS, 3])
                GP.tensor_sub(out=r0[:].bitcast(u32), in0=mb,
                              in1=prod[:].bitcast(u32))
                GP.tensor_mul(out=n1, in0=prod, in1=r0)
                GP.tensor_sub(out=n2, in0=tb, in1=n1)
                GP.tensor_mul(out=r1, in0=r0, in1=n2)
                GP.tensor_mul(out=n3, in0=prod, in1=r1)
                GP.tensor_sub(out=n4, in0=tb, in1=n3)
                GP.tensor_mul(out=w3, in0=r1, in1=n4)

                # scatter through the per-set mask (0-stride broadcasts)
                wblk = cpool.tile([SLOTS, SETS, 3 * K], bf16, tag="wblk")
                for g in range(SETS):
                    GP.tensor_mul(
                        out=wblk[:, g, :].rearrange("p (k c) -> p k c", c=3),
                        in0=w3[:, g, :].unsqueeze(1).broadcast_to([SLOTS, K, 3]),
                        in1=mt[:, g, :].rearrange("p (k c) -> p k c", c=3))

            st = cpool.tile([3 * K, U * HCOL], bf16, tag="stage")
            alphas = []

            # --- hot loop: all sigma matmuls, then all img matmuls (PE
            # stays dense; exp pipelines on ScalarE at 512-col granularity)
            def do_sigma(u):
                alpha = wpool.tile([SLOTS, HCOL], bf16, tag="alpha", name=f"alpha{u}")
                sig = ps_sig_pool.tile([SLOTS, HCOL], f32, tag="sig", name=f"sig{u}")
                nc.tensor.matmul(sig, chilos[u], bt[:], start=True, stop=True)
                S.activation(alpha, sig, Act.Exp, scale=-1.0)
                alphas.append(alpha)

            # Output DMAs stay OFF the scalar queue: its descgen (~1.5us)
            # would block the exp/copy stream behind it.  sync and gpsimd
            # alternate so consecutive units' descgens overlap.
            dmaq = [nc.sync, nc.gpsimd, nc.sync, nc.gpsimd]
            cpeng = [V, S, V, S]  # GpSimd cannot read PSUM

            def do_img(u):
                img = ps_img_pool.tile([3 * K, HCOL], f32, tag="img", name=f"img{u}")
                nc.tensor.matmul(img, wblk[:, u, :], alphas[u], start=True, stop=True)
                # partial tile images are summed and clamped on the host (a
                # split tile's halves meet only post-gather), so the raw
                # accumulator is staged to SBUF bf16 and DMA'd out unclamped
                sl = st[:, HCOL * u:HCOL * (u + 1)]
                if cpeng[u] is S:
                    S.copy(out=sl, in_=img)
                else:
                    cpeng[u].tensor_copy(out=sl, in_=img)
                dmaq[u].dma_start(out=out[:, HCOL * u:HCOL * (u + 1)], in_=sl)

            for u in range(U):
                do_sigma(u)
            for u in range(U):
                do_img(u)

    bass_rust.generate_event_semaphores(nc)
    return nc


def _bin_entries(xyz, cholesky):
    """Host-side routing: which gaussians overlap which 32x32 tile."""
    means = np.tanh(xyz.astype(np.float64))
    cx = 0.5 * W * (means[..., 0] + 1.0)
    cy = 0.5 * H * (means[..., 1] + 1.0)
    chol = cholesky.astype(np.float64) + np.array([0.5, 0.0, 0.5])
    l0, l1, l2 = chol[..., 0], chol[..., 1], chol[..., 2]
    sxx, sxy, syy = l0 * l0, l0 * l1, l1 * l1 + l2 * l2
    tr, det = sxx + syy, sxx * syy - sxy * sxy
    lam = tr / 2 + np.sqrt(np.maximum(tr * tr / 4 - det, 0.0))
    r = np.sqrt(2.0 * SIGMA_CUT * np.maximum(lam, 0.0)) + 1.0

    entries = []  # (frame, ty, tx, index-list)
    for t in range(T):
        x0 = np.clip(((cx[t] - r[t]) // TILE).astype(int), 0, NT - 1)
        x1 = np.clip(((cx[t] + r[t]) // TILE).astype(int), 0, NT - 1)
        y0 = np.clip(((cy[t] - r[t]) // TILE).astype(int), 0, NT - 1)
        y1 = np.clip(((cy[t] + r[t]) // TILE).astype(int), 0, NT - 1)
        buckets = [[[] for _ in range(NT)] for _ in range(NT)]
        for n in range(N):
            for ty in range(y0[n], y1[n] + 1):
                for tx in range(x0[n], x1[n] + 1):
                    buckets[ty][tx].append(n)
        for ty in range(NT):
            for tx in range(NT):
                assert len(buckets[ty][tx]) <= SLOTS, "tile overflow: >128 gaussians"
                if buckets[ty][tx]:
                    entries.append((t, ty, tx, buckets[ty][tx]))
    return entries


def _pack_groups(entries):
    """Pour tiles into 128-slot groups, SPLITTING a tile across groups when a
    group fills (rendering is additive pre-clamp, so partial tile images from
    different groups are summed on the host).  Yields ceil(total/128) groups —
    a perfect pack."""
    groups = [[0, []]]  # [used_slots, [(entry_idx, idx_sublist, slot_start)]]
    for i in range(len(entries)):
        ids = entries[i][3]
        pos = 0
        while pos < len(ids):
            g = groups[-1]
            space = SLOTS - g[0]
            if space == 0:
                groups.append([0, []])
                continue
            take = min(space, len(ids) - pos)
            g[1].append((i, ids[pos:pos + take], g[0]))
            g[0] += take
            pos += take
    return groups


def _ensure_ntff_hook():
    """Provide antenv.axon_hooks (missing in this image) so trace=True works."""
    import sys, types, ctypes, contextlib
    if "antenv.axon_hooks" in sys.modules:
        return
    so_path = "/opt/axon/libaxon_pjrt.so"
    if not os.path.exists(so_path):
        return
    lib = ctypes.CDLL(so_path)
    if not hasattr(lib, "axon_start_nrt_profile"):
        return
    lib.axon_start_nrt_profile.argtypes = [ctypes.POINTER(ctypes.c_int64), ctypes.c_size_t]
    lib.axon_start_nrt_profile.restype = ctypes.c_int64
    lib.axon_stop_nrt_profile.argtypes = [ctypes.c_char_p]
    lib.axon_stop_nrt_profile.restype = ctypes.c_int64

    @contextlib.contextmanager
    def _hook(output_dir, device_ids):
        import jax
        jax.devices()
        if device_ids:
            ids = (ctypes.c_int64 * len(device_ids))(*device_ids)
            rc = lib.axon_start_nrt_profile(ids, len(device_ids))
        else:
            rc = lib.axon_start_nrt_profile(None, 0)
        if rc != 0:
            raise RuntimeError(f"axon_start_nrt_profile rc={rc}")
        try:
            yield
        finally:
            n = lib.axon_stop_nrt_profile(str(output_dir).encode())
            print(f"profile: {n} file(s) written to {output_dir}")

    mod = types.ModuleType("antenv.axon_hooks")
    mod.get_axon_ntff_profile_hook = lambda: _hook
    mod.set_axon_ntff_profile_hook = lambda h: None
    sys.modules["antenv.axon_hooks"] = mod


def kernel(xyz, cholesky, opacity, features_dc):
    from concourse import bass_utils

    xyz = np.asarray(xyz, np.float32)
    cholesky = np.asarray(cholesky, np.float32)
    opacity = np.asarray(opacity, np.float32)
    features_dc = np.asarray(features_dc, np.float32)

    entries = _bin_entries(xyz, cholesky)
    groups = _pack_groups(entries)
    # ~27 packed groups over 8 cores: 4 slot-sets per core, each rendered
    # as one unit of [128 slots x 256 px] (a full 16x16 tile space)
    SETS, U, HCOL = 4, 4, PIX
    assert len(groups) <= SETS * N_CORES, "packing exceeds 4 groups/core"
    while len(groups) < SETS * N_CORES:
        groups.append([0, []])
    K = max((len(gr[1]) for gr in groups), default=1) or 1

    # tile-centered integer basis: exact in bf16.  10 rows
    # [x2 xy y2 x x y y 1 1 1] match the single-product coefficient set;
    # rows 10-19 duplicate rows 0-9 so the hi/lo-stacked [20,128] lhsT
    # contracts both halves of the coefficient split in one matmul pass.
    p = np.arange(HCOL)
    x = (p % TILE - TILE // 2).astype(np.float32)
    y = (p // TILE - TILE // 2).astype(np.float32)
    o = np.ones(HCOL, np.float32)
    b10 = np.stack([x * x, x * y, y * y, x, x, y, y, o, o, o])
    b20 = np.concatenate([b10, b10], axis=0).astype(ml_dtypes.bfloat16)
    ident = np.eye(SLOTS, dtype=ml_dtypes.bfloat16)

    core_sets = [[c + s * N_CORES for s in range(SETS)] for c in range(N_CORES)]

    in_maps = []
    unpack = []  # per core: list of (u, j, t, ty, tx)
    for c in range(N_CORES):
        pm = np.zeros((SLOTS, SETS, 12), np.float32)
        mk = np.zeros((SLOTS, SETS, 3 * K), np.float32)
        um = []
        for s in range(SETS):
            gi = core_sets[c][s]
            for j, (ei, ids, s0) in enumerate(groups[gi][1]):
                t, ty, tx, _ = entries[ei]
                ns = len(ids)
                ids = np.asarray(ids)
                pm[s0:s0 + ns, s, 0:2] = xyz[t, ids]
                pm[s0:s0 + ns, s, 2:5] = cholesky[t, ids]
                pm[s0:s0 + ns, s, 5] = opacity[ids, 0]
                pm[s0:s0 + ns, s, 6:9] = features_dc[ids]
                pm[s0:s0 + ns, s, 9] = tx * TILE + TILE / 2 - 0.5 * W
                pm[s0:s0 + ns, s, 10] = ty * TILE + TILE / 2 - 0.5 * H
                mk[s0:s0 + ns, s, 3 * j:3 * j + 3] = 1.0
                um.append((s, j, t, ty, tx))
        in_maps.append({"params": pm.reshape(SLOTS, SETS * 12),
                        "basis": b20,
                        "msk": mk.reshape(SLOTS, SETS * 3 * K).astype(ml_dtypes.bfloat16),
                        "ident": ident})
        unpack.append(um)

    if (U, SETS, K) not in _CACHE:
        _CACHE[(U, SETS, K)] = _build_nc(U, SETS, K)
    nc = _CACHE[(U, SETS, K)]

    trace = bool(int(os.environ.get("GS_TRACE", "0")))
    if trace:
        _ensure_ntff_hook()
    res = bass_utils.run_bass_kernel_spmd(
        nc, in_maps, core_ids=list(range(N_CORES)), trace=trace)
    kernel.last_result = res

    img = np.zeros((T, 3, H, W), np.float64)
    for c in range(N_CORES):
        o = res.results[c]["out"]
        o = np.asarray(o, np.float64)
        for (u, j, t, ty, tx) in unpack[c]:
            img[t, :, ty * TILE:(ty + 1) * TILE, tx * TILE:(tx + 1) * TILE] += \
                o[3 * j:3 * j + 3, u * HCOL:(u + 1) * HCOL].reshape(3, TILE, TILE)
    return np.clip(img, 0.0, 1.0).astype(np.float32)


# revision 37
# speedup vs baseline: 1.0880x; 1.0880x over previous
"""GaussianImage (Cholesky) renderer on 8 trn2 NeuronCores.

Tile-parallel over the pixel grid: the 256x256 image is cut into 32x32
tiles (64/frame, 128 total).  The host bins gaussians to tiles (pure
routing via a conservative support radius), then pours the tile lists
into sixteen 128-slot "groups", SPLITTING a tile across groups when one
fills -- rendering is an additive accumulation, so partial tile images
are summed on the host, and the final clamp happens post-gather (the
clamp must follow the reduce).  Each core runs exactly 2 groups (slot
sets), each rendered as 2 pixel-half units of [128 slots x 512 px].
Per-core params / masks / outputs are DATA, so the SPMD program is
identical on every core.  All value math runs on device:

  per gaussian slot : tanh / exp-sigmoid / conic / quadratic-basis
                      coeffs split across Scalar(ACT)/Vector/GpSimd
  per slot set      : coeffs hi/lo-split in the FREE dim as bf16, one
                      PE transpose -> stacked [12,128] lhsT, so sigma
                      is ONE matmul with contraction 12 (the hi+lo
                      accumulation rides the contraction dim for free;
                      fp32-level coefficients at bf16 matmul rate)
  per unit          : sigma = chilo(12,128)^T @ basis(12,512)  [TensorE]
                      alpha = Exp(-sigma)           -> bf16    [ScalarE]
                      img   = wblk(128,3K)^T @ alpha(128,512)  [TensorE]
                      out   = bf16 copy to SBUF, DMA out unclamped

Single ACT table set: everything Scalar runs (exp/tanh/square/copy)
lives in the `exp_and_others` table, so exactly one ACT_TABLE_LOAD is
issued, and it loads eagerly during the input DMAs (the baseline paid
two 1.3us loads).  Sigmoids for opacity/color are computed as
1/((1+e^-o)(1+e^-c)) via Exp + Vector reciprocal, folding the
opacity*color product into the same reciprocal.  Input DMAs are split
across the sync (params, basis) and gpsimd (msk, ident) queues so the
params DMA -- which gates all setup math -- issues first; outputs are
bf16 and spread over sync/gpsimd/scalar queues.
"""

import os
import numpy as np
import ml_dtypes

T, N, H, W = 2, 512, 256, 256
TILE = 16
NT = H // TILE          # 16 tiles per axis
N_CORES = 8
SLOTS = 128
PIX = TILE * TILE       # 256
SIGMA_CUT = 8.0         # exp(-8)*opac ~ 2e-4 tail: invisible at 2e-2 tolerance

_CACHE = {}


def _build_nc(U, SETS, K):
    """U pixel-units (512 cols each) over SETS slot-sets per core.

    Unit u reads slot-set u//2 and pixel-half u%2; params, masks and
    outputs are per-core data, so the SPMD program is identical on every
    core while cores render different (gaussian-set, pixel-half) units."""
    import concourse.bass as bass
    import concourse.mybir as mybir
    from concourse.tile import TileContext
    import bass_rust

    f32 = mybir.dt.float32
    bf16 = mybir.dt.bfloat16
    Alu = mybir.AluOpType
    Act = mybir.ActivationFunctionType

    HCOL = PIX              # 256: one unit = one slot-set over a full tile
    NB = 10                 # basis rows: x2 xy y2 x x y y 1 1 1
    nc = bass.Bass("TRN2")
    params = nc.dram_tensor("params", [SLOTS, SETS * 12], f32, kind="ExternalInput")
    basis = nc.dram_tensor("basis", [2 * NB, HCOL], bf16, kind="ExternalInput")
    msk = nc.dram_tensor("msk", [SLOTS, SETS * 3 * K], bf16, kind="ExternalInput")
    ident = nc.dram_tensor("ident", [SLOTS, SLOTS], bf16, kind="ExternalInput")
    out = nc.dram_tensor("out", [3 * K, U * HCOL], bf16, kind="ExternalOutput")

    with TileContext(nc) as tc:
        with tc.tile_pool(name="const", bufs=1) as cpool, \
             tc.tile_pool(name="work", bufs=5) as wpool, \
             tc.tile_pool(name="ps_sig", bufs=4, space="PSUM") as ps_sig_pool, \
             tc.tile_pool(name="ps_img", bufs=4, space="PSUM") as ps_img_pool:

            p3 = cpool.tile([SLOTS, SETS, 12], f32, tag="params")
            bt = cpool.tile([2 * NB, HCOL], bf16, tag="basis")
            mt = cpool.tile([SLOTS, SETS, 3 * K], bf16, tag="msk")
            it = cpool.tile([SLOTS, SLOTS], bf16, tag="ident")
            # Every DMA is shredded into one ring descriptor per partition row
            # (~9ns each, globally dispatched in issue order after a ~770ns
            # ring-kick lag), so params -- which gates ALL setup math -- is
            # issued FIRST and split across the two hwdge queues to halve its
            # descriptor tail.  The rest follows; msk/ident/basis are only
            # consumed ~2-3us later.
            HP = SLOTS // 2
            pview = params[:].rearrange("p (g k) -> p g k", k=12)
            nc.sync.dma_start(out=p3[0:HP], in_=pview[0:HP])
            nc.scalar.dma_start(out=p3[HP:SLOTS], in_=pview[HP:SLOTS])
            nc.sync.dma_start(out=mt, in_=msk[:].rearrange("p (g k) -> p g k", k=3 * K))
            nc.sync.dma_start(out=bt, in_=basis[:])
            # ident rides the gpsimd (softdge) ring, issued LAST so its 128
            # descriptors dispatch after everything else -- and the doorbell
            # pre-arms the gpsimd ring, whose first kick otherwise costs the
            # output DMAs ~600ns at the tail.
            nc.gpsimd.dma_start(out=it, in_=ident[:])

            V = nc.vector
            S = nc.scalar
            GP = nc.gpsimd

            # hf doubles as the Cholesky +0.5 bound and the warm-up input.
            hf = cpool.tile([SLOTS, 1], f32, tag="hf")
            GP.memset(hf, 0.5)
            # warm-up exp: zero-dependency first ACT op.  Every function the
            # Scalar engine runs (exp/tanh/square/copy) is in the
            # exp_and_others table set, so this triggers the ONLY table load
            # of the kernel, overlapping the input DMAs.
            warm = cpool.tile([SLOTS, 1], f32, tag="warm")
            S.activation(warm, hf, Act.Exp)

            def sc(tag):
                return cpool.tile([SLOTS, SETS], f32, tag=tag, name=tag)

            cth = cpool.tile([SLOTS, SETS, NB], f32, tag="coef")

            # --- critical setup chain in natural (early) priority; the
            # off-critical color/opacity block below is pushed to a very LATE
            # priority so the list scheduler never wedges it (w3's reciprocal
            # especially) into an idle slot AHEAD of this chain.
            tx = cpool.tile([SLOTS, SETS, 2], f32, tag="tx")
            t2, t3 = sc("t2"), sc("t3")
            cc = cpool.tile([SLOTS, SETS, 2 * NB], bf16, tag="cc")
            if True:
                # Scalar: tanh centers + the squares gating the det chain
                S.activation(tx, p3[:, :, 0:2], Act.Tanh)
                S.activation(t2, p3[:, :, 3], Act.Square)
                S.activation(t3, p3[:, :, 4], Act.Square, bias=hf)

                # GpSimd: head of the det chain
                a0 = sc("a0")
                GP.tensor_add(out=a0, in0=p3[:, :, 2],
                              in1=hf.broadcast_to([SLOTS, SETS]))
                a1 = p3[:, :, 3]
                t0, t1, v = sc("t0"), sc("t1"), sc("v")
                GP.tensor_mul(out=t0, in0=a0, in1=a0)
                GP.tensor_mul(out=t1, in0=a0, in1=a1)
                GP.tensor_mul(out=v, in0=t1, in1=t1)

                # Vector: centers, det -> rdet -> conic coeffs
                # cx = 0.5*W*(tanh(z)+1); host bakes p9 = tile_cx - 0.5*W so
                # ex = 0.5*W*tanh - p9.
                ex, ey = sc("ex"), sc("ey")
                V.scalar_tensor_tensor(out=ex, in0=tx[:, :, 0], scalar=0.5 * W,
                                       in1=p3[:, :, 9], op0=Alu.mult,
                                       op1=Alu.subtract)
                V.scalar_tensor_tensor(out=ey, in0=tx[:, :, 1], scalar=0.5 * H,
                                       in1=p3[:, :, 10], op0=Alu.mult,
                                       op1=Alu.subtract)
                syy = sc("syy")
                V.tensor_add(out=syy, in0=t2, in1=t3)
                u_, det, rdet = sc("u"), sc("det"), sc("rdet")
                V.tensor_mul(out=u_, in0=t0, in1=syy)
                V.tensor_sub(out=det, in0=u_, in1=v)
                V.reciprocal(out=rdet, in_=det)
                # cth0 = 0.5*ca, cth1 = cb = -sxy/det, cth2 = 0.5*cc
                V.scalar_tensor_tensor(out=cth[:, :, 0], in0=syy, scalar=0.5,
                                       in1=rdet, op0=Alu.mult, op1=Alu.mult)
                V.scalar_tensor_tensor(out=cth[:, :, 1], in0=t1, scalar=-1.0,
                                       in1=rdet, op0=Alu.mult, op1=Alu.mult)
                V.scalar_tensor_tensor(out=cth[:, :, 2], in0=t0, scalar=0.5,
                                       in1=rdet, op0=Alu.mult, op1=Alu.mult)
                # The 10-row basis [x2 xy y2 x x y y 1 1 1] lets every
                # remaining coefficient be ONE fused stt op at depth <= 2 past
                # cth0-2 (instead of the depth-5 m/n accumulation chains); the
                # extra rows ride the matmul contraction dim for free.
                #   x rows: p_a = -ca*ex        p_b = -cb*ey
                #   y rows: p_c = -cc*ey        p_d = -cb*ex
                #   1 rows: q_a = .5*ca*ex^2    q_b = cb*ex*ey  q_c = .5*cc*ey^2
                # (TensorScalarPtr only exists on DVE, so all seven sit on V.)
                V.scalar_tensor_tensor(out=cth[:, :, 3], in0=ex, scalar=-2.0,
                                       in1=cth[:, :, 0], op0=Alu.mult, op1=Alu.mult)
                V.scalar_tensor_tensor(out=cth[:, :, 4], in0=ey, scalar=-1.0,
                                       in1=cth[:, :, 1], op0=Alu.mult, op1=Alu.mult)
                V.scalar_tensor_tensor(out=cth[:, :, 5], in0=ey, scalar=-2.0,
                                       in1=cth[:, :, 2], op0=Alu.mult, op1=Alu.mult)
                V.scalar_tensor_tensor(out=cth[:, :, 6], in0=ex, scalar=-1.0,
                                       in1=cth[:, :, 1], op0=Alu.mult, op1=Alu.mult)
                V.scalar_tensor_tensor(out=cth[:, :, 7], in0=ex, scalar=-0.5,
                                       in1=cth[:, :, 3], op0=Alu.mult, op1=Alu.mult)
                V.scalar_tensor_tensor(out=cth[:, :, 8], in0=ey, scalar=-1.0,
                                       in1=cth[:, :, 6], op0=Alu.mult, op1=Alu.mult)
                V.scalar_tensor_tensor(out=cth[:, :, 9], in0=ey, scalar=-0.5,
                                       in1=cth[:, :, 5], op0=Alu.mult, op1=Alu.mult)

                # hi/lo split IN THE FREE DIM as bf16, then one PE transpose
                # per set yields a stacked [20,128] lhsT: sigma rides the
                # contraction dim (free on the PE) instead of a 2nd matmul.
                V.tensor_copy(out=cc[:, :, 0:NB], in_=cth)
                V.tensor_sub(out=cc[:, :, NB:2 * NB], in0=cth, in1=cc[:, :, 0:NB])

                chilos = []
                for g in range(SETS):
                    tpg = ps_img_pool.tile([2 * NB, SLOTS], bf16, tag="img",
                                           name=f"tp{g}")
                    nc.tensor.transpose(tpg, cc[:, g, :], it)
                    chilo = cpool.tile([2 * NB, SLOTS], bf16, tag=f"chilo{g}",
                                       name=f"chilo{g}")
                    # copies alternate engines so consecutive sets land in
                    # parallel; set 0 rides Vector (its copy is ~150ns
                    # faster) because it gates the first sigma matmul.
                    if g % 2 == 0:
                        V.tensor_copy(out=chilo, in_=tpg)
                    else:
                        S.copy(out=chilo, in_=tpg)
                    chilos.append(chilo)

            # --- off-critical-path setup at VERY LATE priority (loses every
            # ready-tie, still fills genuinely idle slots): colors*opacity
            # without the sigmoid table: sigmoid(o)*sigmoid(c) =
            # 1/((1+e^-o)(1+e^-c)) -- one reciprocal.
            with tc.high_priority(offset=-100000):
                eo = sc("eo")
                S.activation(eo, p3[:, :, 5], Act.Exp, scale=-1.0)
                ew = cpool.tile([SLOTS, SETS, 3], f32, tag="ew")
                S.activation(ew, p3[:, :, 6:9], Act.Exp, scale=-1.0)
                deno = sc("deno")
                GP.tensor_scalar_add(deno, eo, 1.0)
                denw = cpool.tile([SLOTS, SETS, 3], f32, tag="denw")
                GP.tensor_scalar_add(denw, ew, 1.0)
                prod = cpool.tile([SLOTS, SETS, 3], f32, tag="prodw")
                GP.tensor_mul(out=prod, in0=denw,
                              in1=deno.unsqueeze(2).broadcast_to([SLOTS, SETS, 3]))
                # 1/prod on GpSimd (bit-trick seed + 2 Newton steps, rel err
                # ~1e-6): keeps the reciprocal OFF Vector, whose static
                # schedule otherwise stalls the det chain waiting for prod.
                u32 = mybir.dt.uint32
                magic = cpool.tile([SLOTS, 1], u32, tag="magic")
                GP.memset(magic, 0x7EF127EA)
                two = cpool.tile([SLOTS, 1], f32, tag="two")
                GP.memset(two, 2.0)
                w3 = cpool.tile([SLOTS, SETS, 3], f32, tag="w3")
                r0 = cpool.tile([SLOTS, SETS, 3], f32, tag="w3r0")
                r1 = cpool.tile([SLOTS, SETS, 3], f32, tag="w3r1")
                n1 = cpool.tile([SLOTS, SETS, 3], f32, tag="w3n1")
                n2 = cpool.tile([SLOTS, SETS, 3], f32, tag="w3n2")
                n3 = cpool.tile([SLOTS, SETS, 3], f32, tag="w3n3")
                n4 = cpool.tile([SLOTS, SETS, 3], f32, tag="w3n4")
                mb = magic.unsqueeze(2).broadcast_to([SLOTS, SETS, 3])
                tb = two.unsqueeze(2).broadcast_to([SLOTS, SETS, 3])
                GP.tensor_sub(out=r0[:].bitcast(u32), in0=mb,
                              in1=prod[:].bitcast(u32))
                GP.tensor_mul(out=n1, in0=prod, in1=r0)
                GP.tensor_sub(out=n2, in0=tb, in1=n1)
                GP.tensor_mul(out=r1, in0=r0, in1=n2)
                GP.tensor_mul(out=n3, in0=prod, in1=r1)
                GP.tensor_sub(out=n4, in0=tb, in1=n3)
                GP.tensor_mul(out=w3, in0=r1, in1=n4)

                # scatter through the per-set mask (0-stride broadcasts)
                wblk = cpool.tile([SLOTS, SETS, 3 * K], bf16, tag="wblk")
                for g in range(SETS):
                    GP.tensor_mul(
                        out=wblk[:, g, :].rearrange("p (k c) -> p k c", c=3),
                        in0=w3[:, g, :].unsqueeze(1).broadcast_to([SLOTS, K, 3]),
                        in1=mt[:, g, :].rearrange("p (k c) -> p k c", c=3))

            st = cpool.tile([3 * K, U * HCOL], bf16, tag="stage")
            alphas = []

            # --- hot loop: all sigma matmuls, then all img matmuls (PE
            # stays dense; exp pipelines on ScalarE at 512-col granularity)
            def do_sigma(u):
                alpha = wpool.tile([SLOTS, HCOL], bf16, tag="alpha", name=f"alpha{u}")
                sig = ps_sig_pool.tile([SLOTS, HCOL], f32, tag="sig", name=f"sig{u}")
                nc.tensor.matmul(sig, chilos[u], bt[:], start=True, stop=True)
                S.activation(alpha, sig, Act.Exp, scale=-1.0)
                alphas.append(alpha)

            # Output DMAs stay OFF the scalar queue: its descgen (~1.5us)
            # would block the exp/copy stream behind it.  sync and gpsimd
            # alternate so consecutive units' descgens overlap.
            dmaq = [nc.sync, nc.gpsimd, nc.sync, nc.gpsimd]
            cpeng = [V, S, V, S]  # GpSimd cannot read PSUM

            def do_img(u):
                img = ps_img_pool.tile([3 * K, HCOL], f32, tag="img", name=f"img{u}")
                nc.tensor.matmul(img, wblk[:, u, :], alphas[u], start=True, stop=True)
                # partial tile images are summed and clamped on the host (a
                # split tile's halves meet only post-gather), so the raw
                # accumulator is staged to SBUF bf16 and DMA'd out unclamped
                sl = st[:, HCOL * u:HCOL * (u + 1)]
                if u == U - 1:
                    # the LAST copy gates the tail: split across S and V
                    HH = HCOL // 2
                    S.copy(out=sl[:, 0:HH], in_=img[:, 0:HH])
                    V.tensor_copy(out=sl[:, HH:HCOL], in_=img[:, HH:HCOL])
                elif cpeng[u] is S:
                    S.copy(out=sl, in_=img)
                else:
                    cpeng[u].tensor_copy(out=sl, in_=img)
                dmaq[u].dma_start(out=out[:, HCOL * u:HCOL * (u + 1)], in_=sl)

            for u in range(U):
                do_sigma(u)
            for u in range(U):
                do_img(u)

    bass_rust.generate_event_semaphores(nc)
    return nc


def _bin_entries(xyz, cholesky):
    """Host-side routing: which gaussians overlap which 32x32 tile."""
    means = np.tanh(xyz.astype(np.float64))
    cx = 0.5 * W * (means[..., 0] + 1.0)
    cy = 0.5 * H * (means[..., 1] + 1.0)
    chol = cholesky.astype(np.float64) + np.array([0.5, 0.0, 0.5])
    l0, l1, l2 = chol[..., 0], chol[..., 1], chol[..., 2]
    sxx, sxy, syy = l0 * l0, l0 * l1, l1 * l1 + l2 * l2
    tr, det = sxx + syy, sxx * syy - sxy * sxy
    lam = tr / 2 + np.sqrt(np.maximum(tr * tr / 4 - det, 0.0))
    r = np.sqrt(2.0 * SIGMA_CUT * np.maximum(lam, 0.0)) + 1.0

    entries = []  # (frame, ty, tx, index-list)
    for t in range(T):
        x0 = np.clip(((cx[t] - r[t]) // TILE).astype(int), 0, NT - 1)
        x1 = np.clip(((cx[t] + r[t]) // TILE).astype(int), 0, NT - 1)
        y0 = np.clip(((cy[t] - r[t]) // TILE).astype(int), 0, NT - 1)
        y1 = np.clip(((cy[t] + r[t]) // TILE).astype(int), 0, NT - 1)
        buckets = [[[] for _ in range(NT)] for _ in range(NT)]
        for n in range(N):
            for ty in range(y0[n], y1[n] + 1):
                for tx in range(x0[n], x1[n] + 1):
                    buckets[ty][tx].append(n)
        for ty in range(NT):
            for tx in range(NT):
                assert len(buckets[ty][tx]) <= SLOTS, "tile overflow: >128 gaussians"
                if buckets[ty][tx]:
                    entries.append((t, ty, tx, buckets[ty][tx]))
    return entries


def _pack_groups(entries):
    """Pour tiles into 128-slot groups, SPLITTING a tile across groups when a
    group fills (rendering is additive pre-clamp, so partial tile images from
    different groups are summed on the host).  Yields ceil(total/128) groups —
    a perfect pack."""
    groups = [[0, []]]  # [used_slots, [(entry_idx, idx_sublist, slot_start)]]
    for i in range(len(entries)):
        ids = entries[i][3]
        pos = 0
        while pos < len(ids):
            g = groups[-1]
            space = SLOTS - g[0]
            if space == 0:
                groups.append([0, []])
                continue
            take = min(space, len(ids) - pos)
            g[1].append((i, ids[pos:pos + take], g[0]))
            g[0] += take
            pos += take
    return groups


def _ensure_ntff_hook():
    """Provide antenv.axon_hooks (missing in this image) so trace=True works."""
    import sys, types, ctypes, contextlib
    if "antenv.axon_hooks" in sys.modules:
        return
    so_path = "/opt/axon/libaxon_pjrt.so"
    if not os.path.exists(so_path):
        return
    lib = ctypes.CDLL(so_path)
    if not hasattr(lib, "axon_start_nrt_profile"):
        return
    lib.axon_start_nrt_profile.argtypes = [ctypes.POINTER(ctypes.c_int64), ctypes.c_size_t]
    lib.axon_start_nrt_profile.restype = ctypes.c_int64
    lib.axon_stop_nrt_profile.argtypes = [ctypes.c_char_p]
    lib.axon_stop_nrt_profile.restype = ctypes.c_int64

    @contextlib.contextmanager
    def _hook(output_dir, device_ids):
        import jax
        jax.devices()
        if device_ids:
            ids = (ctypes.c_int64 * len(device_ids))(*device_ids)
            rc = lib.axon_start_nrt_profile(ids, len(device_ids))
        else:
            rc = lib.axon_start_nrt_profile(None, 0)
        if rc != 0:
            raise RuntimeError(f"axon_start_nrt_profile rc={rc}")
        try:
            yield
        finally:
            n = lib.axon_stop_nrt_profile(str(output_dir).encode())
            print(f"profile: {n} file(s) written to {output_dir}")

    mod = types.ModuleType("antenv.axon_hooks")
    mod.get_axon_ntff_profile_hook = lambda: _hook
    mod.set_axon_ntff_profile_hook = lambda h: None
    sys.modules["antenv.axon_hooks"] = mod


def kernel(xyz, cholesky, opacity, features_dc):
    from concourse import bass_utils

    xyz = np.asarray(xyz, np.float32)
    cholesky = np.asarray(cholesky, np.float32)
    opacity = np.asarray(opacity, np.float32)
    features_dc = np.asarray(features_dc, np.float32)

    entries = _bin_entries(xyz, cholesky)
    groups = _pack_groups(entries)
    # ~27 packed groups over 8 cores: 4 slot-sets per core, each rendered
    # as one unit of [128 slots x 256 px] (a full 16x16 tile space)
    SETS, U, HCOL = 4, 4, PIX
    assert len(groups) <= SETS * N_CORES, "packing exceeds 4 groups/core"
    while len(groups) < SETS * N_CORES:
        groups.append([0, []])
    K = max((len(gr[1]) for gr in groups), default=1) or 1

    # tile-centered integer basis: exact in bf16.  10 rows
    # [x2 xy y2 x x y y 1 1 1] match the single-product coefficient set;
    # rows 10-19 duplicate rows 0-9 so the hi/lo-stacked [20,128] lhsT
    # contracts both halves of the coefficient split in one matmul pass.
    p = np.arange(HCOL)
    x = (p % TILE - TILE // 2).astype(np.float32)
    y = (p // TILE - TILE // 2).astype(np.float32)
    o = np.ones(HCOL, np.float32)
    b10 = np.stack([x * x, x * y, y * y, x, x, y, y, o, o, o])
    b20 = np.concatenate([b10, b10], axis=0).astype(ml_dtypes.bfloat16)
    ident = np.eye(SLOTS, dtype=ml_dtypes.bfloat16)

    core_sets = [[c + s * N_CORES for s in range(SETS)] for c in range(N_CORES)]

    in_maps = []
    unpack = []  # per core: list of (u, j, t, ty, tx)
    for c in range(N_CORES):
        pm = np.zeros((SLOTS, SETS, 12), np.float32)
        mk = np.zeros((SLOTS, SETS, 3 * K), np.float32)
        um = []
        for s in range(SETS):
            gi = core_sets[c][s]
            for j, (ei, ids, s0) in enumerate(groups[gi][1]):
                t, ty, tx, _ = entries[ei]
                ns = len(ids)
                ids = np.asarray(ids)
                pm[s0:s0 + ns, s, 0:2] = xyz[t, ids]
                pm[s0:s0 + ns, s, 2:5] = cholesky[t, ids]
                pm[s0:s0 + ns, s, 5] = opacity[ids, 0]
                pm[s0:s0 + ns, s, 6:9] = features_dc[ids]
                pm[s0:s0 + ns, s, 9] = tx * TILE + TILE / 2 - 0.5 * W
                pm[s0:s0 + ns, s, 10] = ty * TILE + TILE / 2 - 0.5 * H
                mk[s0:s0 + ns, s, 3 * j:3 * j + 3] = 1.0
                um.append((s, j, t, ty, tx))
        in_maps.append({"params": pm.reshape(SLOTS, SETS * 12),
                        "basis": b20,
                        "msk": mk.reshape(SLOTS, SETS * 3 * K).astype(ml_dtypes.bfloat16),
                        "ident": ident})
        unpack.append(um)

    if (U, SETS, K) not in _CACHE:
        _CACHE[(U, SETS, K)] = _build_nc(U, SETS, K)
    nc = _CACHE[(U, SETS, K)]

    trace = bool(int(os.environ.get("GS_TRACE", "0")))
    if trace:
        _ensure_ntff_hook()
    res = bass_utils.run_bass_kernel_spmd(
        nc, in_maps, core_ids=list(range(N_CORES)), trace=trace)
    kernel.last_result = res

    img = np.zeros((T, 3, H, W), np.float64)
    for c in range(N_CORES):
        o = res.results[c]["out"]
        o = np.asarray(o, np.float64)
        for (u, j, t, ty, tx) in unpack[c]:
            img[t, :, ty * TILE:(ty + 1) * TILE, tx * TILE:(tx + 1) * TILE] += \
                o[3 * j:3 * j + 3, u * HCOL:(u + 1) * HCOL].reshape(3, TILE, TILE)
    return np.clip(img, 0.0, 1.0).astype(np.float32)
